# revision 34
# baseline (speedup 1.0000x reference)
"""Trainium2 Bass kernel for a dense transformer encoder layer.

Contract: kernel(**inputs) takes FULL unsharded inputs (as produced by the
problem's setup_inputs) and returns the FULL output [B, L, D] float32.

Sharding: 8 cores, data-parallel over batch (4) x sequence-split (2).
Core c handles batch b=c//2, sequence half h=c%2 (1024 query rows), but
computes K/V over the full 2048 keys of its batch item (keys are rotated so
each core's own rows come first -> one identical SPMD program, per-core data
only). No collectives.

Host prep (layout only): fold ln1_g/ln1_b into Wqkv/bqkv, ln2_g/ln2_b into
W1/b1, fold 1/sqrt(dh) into Wq/bq, de-interleave Wqkv into [Q|K] (feature-
major outputs) and V (row-major output), cast weights to bf16.

v2 on-chip dataflow per core (all matmuls bf16 with fp32 PSUM accumulate):
  Phase A (per 128-row tile): LN1 (bn_stats) -> PE-transpose -> lnT, then V
    matmuls for that tile immediately (keeps HAM clock warm, overlaps DVE LN).
  QK proj for all 6 head pairs -> qT_all/kT_all resident in SBUF.
  Attention, query-chunk (lch=512 rows) outer, head-pair inner:
    per round of 2 key tiles: 4 score matmuls issued A,B,A,B -- heads A/B use
    PE row-groups {0,1} vs {2,3} (K=64), so pairs run CONCURRENTLY on the
    array (2x scores throughput). exp batched N=1024 per head on ScalarE.
    attnV via ones-augmented V (M=65) accumulating [65,512] per head.
    1/sumexp via DVE reciprocal_approx_fast (5x faster than reciprocal).
  After lch=0 attention: out-proj + residual + LN2 + FFN for those 512 rows
    are EMITTED next, so the Tile scheduler fills lch=1's exp-wait PE gaps
    with FFN/proj matmuls. W2 is SBUF-resident (loaded once) so FFN2 needs
    only 2 PSUM banks, leaving the attention 6-bank working set intact.
PSUM budget: scores quad 4 banks + attnV pair 2 banks + general pool 2.
"""

import numpy as np
import ml_dtypes

B, L, D, H, I = 4, 2048, 768, 12, 3072
DH = D // H            # 64
P = 128
LQ = L // 2            # 1024 query rows per core
NCORES = 8
EPS = 1e-5

KD = D // P            # 6   k-subtiles over D
KI = I // P            # 24  k-subtiles over I
NT = L // P            # 16  key tiles
NTQ = LQ // P          # 8   query tiles
NPAIR = H // 2         # 6   head pairs
VW = H * 80            # 960: fp8 vaug, per head 64 V cols + ones col + 15 pad

_CACHE = {}


def _bf16(a):
    return np.ascontiguousarray(np.asarray(a, np.float32).astype(ml_dtypes.bfloat16))


def _f32(a):
    return np.ascontiguousarray(np.asarray(a, np.float32))


def _pm(vec, k):
    """[k*128] -> [128, k] partition-major."""
    return np.ascontiguousarray(np.asarray(vec, np.float32).reshape(k, P).T)


def _wpm(w, k):
    """[k*128, M] -> [128, k, M] partition-major lhsT/rhs layout."""
    w = np.asarray(w)
    return np.ascontiguousarray(w.reshape(k, P, w.shape[1]).transpose(1, 0, 2))


def build(use_mask=False):
    import concourse.bass as bass
    import concourse.mybir as mybir
    import concourse.tile as tile
    from concourse import bacc
    from concourse.bass import ts
    from concourse.masks import make_identity
    from contextlib import ExitStack

    f32 = mybir.dt.float32
    bf16 = mybir.dt.bfloat16
    f8 = mybir.dt.float8e4
    PM = mybir.MatmulPerfMode
    AF = mybir.ActivationFunctionType
    OP = mybir.AluOpType

    nc = bacc.Bacc(None, target_bir_lowering=False, debug=False)

    # ---- DRAM I/O ----------------------------------------------------------
    x_d = nc.dram_tensor("xloc", [NT, P, D], f32, kind="ExternalInput")
    mb_d = nc.dram_tensor("mbias", [P, NT], f32, kind="ExternalInput")
    wqk_d = nc.dram_tensor("wqk", [P, KD, 2 * D], bf16, kind="ExternalInput")
    bqk_d = nc.dram_tensor("bqk", [P, 2 * KD], f32, kind="ExternalInput")
    wv_d = nc.dram_tensor("wv", [P, KD, D], bf16, kind="ExternalInput")
    wo_d = nc.dram_tensor("wo", [P, KD, D], bf16, kind="ExternalInput")
    w1_d = nc.dram_tensor("w1", [P, KD, I], bf16, kind="ExternalInput")
    b1_d = nc.dram_tensor("b1", [P, KI], f32, kind="ExternalInput")
    w2_d = nc.dram_tensor("w2", [P, KI, D], bf16, kind="ExternalInput")
    b2_d = nc.dram_tensor("b2", [1, D], bf16, kind="ExternalInput")
    x8_d = nc.dram_tensor("x8", [4, P, D], bf16, kind="ExternalInput")
    out_d = nc.dram_tensor("out", [NTQ, P, D], f32, kind="ExternalOutput")
    scr_d = nc.dram_tensor("warm_scr", [P, P], f32)

    with ExitStack() as ctx:
        tc = ctx.enter_context(tile.TileContext(nc))
        # PSUM: quad (4 banks) for paired scores, pvp (2) for attnV
        # accumulators, g (2) for everything else (V/QK/O/FFN/transposes).
        quadp = ctx.enter_context(tc.tile_pool(name="quadp", bufs=2, space="PSUM"))
        pvpp = ctx.enter_context(tc.tile_pool(name="pvpp", bufs=1, space="PSUM"))
        gp = ctx.enter_context(tc.tile_pool(name="gp", bufs=2, space="PSUM"))
        const = ctx.enter_context(tc.tile_pool(name="const", bufs=1))
        wres = ctx.enter_context(tc.tile_pool(name="wres", bufs=1))
        w1p = ctx.enter_context(tc.tile_pool(name="w1p", bufs=2))
        kvp = ctx.enter_context(tc.tile_pool(name="kvp", bufs=1))
        qkt = ctx.enter_context(tc.tile_pool(name="qkt", bufs=1))
        lnu = ctx.enter_context(tc.tile_pool(name="lnu", bufs=1))
        expp = ctx.enter_context(tc.tile_pool(name="expp", bufs=4))
        xrp = ctx.enter_context(tc.tile_pool(name="xrp", bufs=2))
        tp = ctx.enter_context(tc.tile_pool(name="tp", bufs=2))
        # released mid-build to make room for the resident W2
        xp = tc.alloc_tile_pool(name="xp", bufs=2)
        wstr = tc.alloc_tile_pool(name="wstr", bufs=3)

        nname = [0]

        def gtile(cols=512, dt=f32):
            nname[0] += 1
            return gp.tile([P, cols], dt, tag="g", name=f"g{nname[0]}")

        # first x tile pair + V weights go to the DMA queue ahead of the
        # slow [1,D]->[P,D] broadcast const loads (the first LN was
        # otherwise stuck behind ~12us of replication DMA).
        xt0 = xp.tile([P, 2, D], f32, tag="xl", name="xt0")
        nc.sync.dma_start(xt0[:, 0, :], x_d[0])
        nc.sync.dma_start(xt0[:, 1, :], x_d[1])
        wv_early = wres.tile([P, KD, D], bf16, tag="wow", name="wv_early")
        nc.sync.dma_start(wv_early[:], wv_d[:])

        # ---- constants -----------------------------------------------------
        ident = const.tile([P, P], bf16, tag="ident")
        make_identity(nc, ident)
        epst = const.tile([P, 1], f32, tag="eps")
        nc.vector.memset(epst, EPS)
        u32 = mybir.dt.uint32
        shift1 = const.tile([P, 1], u32, tag="sh1")
        nc.vector.memset(shift1, 1)
        magic = const.tile([P, 4], u32, tag="magic")
        nc.vector.memset(magic, 0x5F3759DF)
        neg2 = const.tile([P, 1], f32, tag="neg2")
        nc.vector.memset(neg2, -2.0)
        mbias = const.tile([P, NT], f32, tag="mb")
        nc.sync.dma_start(mbias[:], mb_d[:])
        bqk_sb = const.tile([P, 2 * KD], f32, tag="bqk")
        nc.sync.dma_start(bqk_sb[:], bqk_d[:])
        b1_sb = const.tile([P, KI], f32, tag="b1")
        nc.sync.dma_start(b1_sb[:], b1_d[:])
        b2_row = const.tile([1, D], bf16, tag="b2r")
        nc.sync.dma_start(b2_row[:], b2_d[:])
        b2_sb = const.tile([P, D], bf16, tag="b2")
        nc.gpsimd.partition_broadcast(b2_sb[:], b2_row[:])

        # persistent activations
        lnT = lnu.tile([P, KD, L], bf16, tag="lnu")        # [768, 2048] transposed LN1
        vaug = kvp.tile([P, NT, VW], f8, tag="vo")         # fp8 V row-major + ones cols
        aoT = kvp.tile([P, KD, LQ], bf16, tag="aoT")       # attn out, feature-major
        out1 = kvp.tile([P, NTQ, D], bf16, tag="out1")     # attn residual stream
        ln2T = kvp.tile([P, KD, LQ], bf16, tag="ln2T")
        qT_all = qkt.tile([P, NPAIR, LQ], bf16, tag="qTa")
        kT_all = qkt.tile([P, NPAIR, L], bf16, tag="kTa")

        def layernorm(dst_bf16, src, stats_tag):
            """dst = (src - mean)/sqrt(var+eps) over free dim 768."""
            view = src.rearrange("p (a b) -> p a b", b=256)
            stats = tp.tile([P, 3, 6], f32, tag=stats_tag + "s")
            mv = tp.tile([P, 2], f32, tag=stats_tag + "m")
            for i in range(3):
                nc.vector.bn_stats(out=stats[:, i, :], in_=view[:, i, :])
            nc.vector.bn_aggr(out=mv[:], in_=stats[:])
            # mv[:,1] = 1/sqrt(var+eps)
            nc.scalar.activation(out=mv[:, 1:2], in_=mv[:, 1:2], func=AF.Sqrt,
                                 bias=epst[:], scale=1.0)
            nc.vector.reciprocal(out=mv[:, 1:2], in_=mv[:, 1:2])
            nc.vector.tensor_scalar(out=dst_bf16, in0=src,
                                    scalar1=mv[:, 0:1], scalar2=mv[:, 1:2],
                                    op0=OP.subtract, op1=OP.mult)

        def transpose_128(dst, src_bf16):
            """dst[128,128] (sbuf bf16) = src.T via PE."""
            nname[0] += 1
            pt = gp.tile([P, P], bf16, tag="g", name=f"pt{nname[0]}")
            nc.tensor.transpose(pt[:], src_bf16, ident[:])
            nc.vector.tensor_copy(out=dst, in_=pt[:])

        # ---- Phase A: LN1 + transpose -> lnT, V per tile -------------------
        wv_sb = wv_early
        vview = vaug.rearrange("p t (h c) -> p t h c", c=80)
        nc.vector.memset(vview[:, :, :, DH : 80], 0.0)
        nc.vector.memset(vview[:, :, :, DH : DH + 1], 1.0)

        def v_tile(t):
            for ncol in range(2):
                pv = gtile(384)
                for k in range(KD):
                    nc.tensor.matmul(pv[:, :384], lnT[:, k, ts(t, P)],
                                     wv_sb[:, k, ts(ncol, 384)],
                                     start=(k == 0), stop=(k == KD - 1))
                dst = vview[:, t, 6 * ncol : 6 * ncol + 6, 0:DH]
                src = pv[:, :384].rearrange("p (h c) -> p h c", c=DH)
                nc.vector.tensor_copy(out=dst, in_=src)

        # HAM warm-up: ~3.5us of real matmuls first thing flips the PE clock
        # gate to 8/8 before the transposes (HAM-invisible) start.
        wps = gtile(P)
        for w in range(40):
            nc.tensor.matmul(wps[:, 0:P], ident[:], ident[:],
                             start=(w == 0), stop=(w == 39))
        wsb = tp.tile([P, P], f32, tag="wsb")
        nc.vector.tensor_copy(out=wsb[:], in_=wps[:, 0:P])
        nc.sync.dma_start(scr_d[:], wsb[:])

        for tpair in range(NT // 2):
            if tpair == 0:
                xt = xt0
            else:
                xt = xp.tile([P, 2, D], f32, tag="xl")
                nc.sync.dma_start(xt[:], x_d[2 * tpair : 2 * tpair + 2].rearrange("t p d -> p t d"))
            for s in range(2):
                t = 2 * tpair + s
                lnbf = tp.tile([P, D], bf16, tag="lnbf")
                layernorm(lnbf[:], xt[:, s, :], "ln1")
                for j in range(KD):
                    transpose_128(lnT[:, j, ts(t, P)], lnbf[:, ts(j, P)])
                v_tile(t)

        # ---- QK projection for one pair -> resident qT_all/kT_all ----------
        def qk_proj(j):
            wqkj = wstr.tile([P, KD, 2 * P], bf16, tag="wqkj")
            nc.sync.dma_start(wqkj[:, :, 0:P], wqk_d[:, :, ts(j, P)])
            nc.sync.dma_start(wqkj[:, :, P : 2 * P], wqk_d[:, :, D + j * P : D + (j + 1) * P])
            for lch in range(2):
                pq = gtile()
                for k in range(KD):
                    nc.tensor.matmul(pq[:], wqkj[:, k, 0:P], lnT[:, k, ts(lch, 512)],
                                     start=(k == 0), stop=(k == KD - 1))
                nc.vector.tensor_scalar(out=qT_all[:, j, ts(lch, 512)], in0=pq[:],
                                        scalar1=bqk_sb[:, j : j + 1], scalar2=None,
                                        op0=OP.add)
            for nch in range(4):
                pk = gtile()
                for k in range(KD):
                    nc.tensor.matmul(pk[:], wqkj[:, k, P : 2 * P], lnT[:, k, ts(nch, 512)],
                                     start=(k == 0), stop=(k == KD - 1))
                nc.vector.tensor_scalar(out=kT_all[:, j, ts(nch, 512)], in0=pk[:],
                                        scalar1=bqk_sb[:, KD + j : KD + j + 1],
                                        scalar2=None, op0=OP.add)

        # ---- attention for (pair j, query chunk lch) -----------------------
        # One round = one key tile: both heads' score matmuls (concurrent on
        # PE row-groups {0,1}/{2,3}) land in one 2-bank quad tile, and a
        # single N=1024 exp covers both heads. quadp bufs=2 double-buffers
        # rounds so scores(r+1) only waits on exp(r-1) -- the exp stream on
        # ScalarE runs gap-free.
        def attention(j, lch):
            pvp = pvpp.tile([P, 2, 512], f32, tag="pvp", name=f"pv{j}_{lch}")
            wd = gtile(P) if lch == 0 else None
            for mtp in range(NT // 2):
                if lch == 0 and 1 <= mtp <= 6:
                    for w in range(8):
                        nc.tensor.matmul(wd[:, 0:P], ident[:], ident[:],
                                         start=(mtp == 1 and w == 0),
                                         stop=(mtp == 6 and w == 7))
                expT2 = expp.tile([P, 2, 2, 512], f8, tag="expT",
                                  name=f"ex{j}_{lch}_{mtp}")
                for s in range(2):
                    mt = 2 * mtp + s
                    dq = quadp.tile([P, 2, 512], f32, tag="quad",
                                    name=f"qd{j}_{lch}_{mt}")
                    for hh in range(2):
                        r = hh * 64
                        nc.tensor.matmul(dq[:, hh, :],
                                         kT_all[r : r + 64, j, ts(mt, P)],
                                         qT_all[r : r + 64, j, ts(lch, 512)],
                                         start=True, stop=True)
                    # exp(s - 2): the -2 keeps exp within fp8e4 range; the
                    # softmax ratio is shift-invariant.
                    if use_mask:
                        for hh in range(2):
                            nc.scalar.activation(out=expT2[:, s, hh, :],
                                                 in_=dq[:, hh, :], func=AF.Exp,
                                                 bias=mbias[:, mt : mt + 1],
                                                 scale=1.0)
                    else:
                        nc.scalar.activation(out=expT2[:, s, :, :], in_=dq[:],
                                             func=AF.Exp, bias=neg2[:])
                for hh in range(2):
                    h = 2 * j + hh
                    nc.tensor.matmul(
                        pvp[0:80, hh, :],
                        vaug[:, 2 * mtp : 2 * mtp + 2, 80 * h : 80 * h + 80],
                        expT2[:, :, hh, :],
                        perf_mode=PM.DoubleRow,
                        start=(mtp == 0), stop=(mtp == NT // 2 - 1))
            if lch == 0:
                wsb2 = tp.tile([P, P], f32, tag="wsb")
                nc.vector.tensor_copy(out=wsb2[:], in_=wd[:, 0:P])
                nc.sync.dma_start(scr_d[:], wsb2[:])
            for hh in range(2):
                r = hh * 64
                rr0 = tp.tile([1, 512], f32, tag="rr0", bufs=1)
                nc.vector.tensor_copy(out=rr0[:], in_=pvp[DH : DH + 1, hh, :])
                rr = tp.tile([1, 512], f32, tag="rr", bufs=1)
                nc.vector.reciprocal_approx_fast(out=rr[:], in_=rr0[:])
                rrb = tp.tile([64, 512], f32, tag="rrb", bufs=2)
                nc.gpsimd.partition_broadcast(rrb[:], rr[:])
                nc.vector.tensor_tensor(out=aoT[r : r + 64, j, ts(lch, 512)],
                                        in0=pvp[0:DH, hh, :],
                                        in1=rrb[:], op=OP.mult)

        # ---- Phase D: out-proj + residual + LN2 + transpose (4 tiles) ------
        # LN2 rstd is computed on DVE only (fast-inverse-sqrt bit trick + 2
        # Newton steps, batched over the 4 tiles) so no ScalarE table switch
        # lands inside the attention exp stream.
        def phase_d(lch, wo_sb):
            mv4 = tp.tile([P, 4, 2], f32, tag="mv4", name=f"mv4_{lch}")
            for tt in range(4):
                t = lch * 4 + tt
                if lch == 1:
                    xr = x8_sb[:, tt, :]
                else:
                    xr = xrp.tile([P, D], f32, tag="xl")
                    nc.sync.dma_start(xr[:], x_d[t].rearrange("p d -> p d"))
                for ncol in range(2):
                    po = gtile(384)
                    for k in range(KD):
                        nc.tensor.matmul(po[:, :384], aoT[:, k, ts(t, P)],
                                         wo_sb[:, k, ts(ncol, 384)],
                                         start=(k == 0), stop=(k == KD - 1))
                    nc.vector.tensor_tensor(out=out1[:, t, ts(ncol, 384)],
                                            in0=po[:, :384],
                                            in1=xr[:, ts(ncol, 384)], op=OP.add)
                view = out1[:, t, :].rearrange("p (a b) -> p a b", b=256)
                stats = tp.tile([P, 3, 6], f32, tag="ln2s")
                for i in range(3):
                    nc.vector.bn_stats(out=stats[:, i, :], in_=view[:, i, :])
                nc.vector.bn_aggr(out=mv4[:, tt, :], in_=stats[:])
            # rstd for all 4 tiles: r = rsqrt(var + eps) without ScalarE
            vr = tp.tile([P, 4], f32, tag="vr", name=f"vr{lch}")
            nc.vector.tensor_scalar(out=vr[:], in0=mv4[:, :, 1],
                                    scalar1=epst[:], scalar2=None, op0=OP.add)
            y0 = tp.tile([P, 4], f32, tag="y0", name=f"y0{lch}")
            nc.vector.tensor_scalar(out=y0.bitcast(u32)[:], in0=vr.bitcast(u32)[:],
                                    scalar1=shift1[:], scalar2=None,
                                    op0=OP.logical_shift_right)
            nc.vector.tensor_tensor(out=y0.bitcast(u32)[:], in0=magic[:],
                                    in1=y0.bitcast(u32)[:], op=OP.subtract)
            for it in range(2):
                aa = tp.tile([P, 4], f32, tag="nsq", name=f"nsq{lch}_{it}")
                nc.vector.tensor_tensor(out=aa[:], in0=y0[:], in1=y0[:], op=OP.mult)
                nc.vector.tensor_tensor(out=aa[:], in0=aa[:], in1=vr[:], op=OP.mult)
                nc.vector.tensor_scalar(out=aa[:], in0=aa[:],
                                        scalar1=-0.5, scalar2=1.5,
                                        op0=OP.mult, op1=OP.add)
                nc.vector.tensor_tensor(out=y0[:], in0=y0[:], in1=aa[:], op=OP.mult)
            for tt in range(4):
                t = lch * 4 + tt
                lnbf = tp.tile([P, D], bf16, tag="lnbf")
                nc.vector.tensor_scalar(out=lnbf[:], in0=out1[:, t, :],
                                        scalar1=mv4[:, tt, 0:1], scalar2=y0[:, tt : tt + 1],
                                        op0=OP.subtract, op1=OP.mult)
                for k in range(KD):
                    transpose_128(ln2T[:, k, ts(t, P)], lnbf[:, ts(k, P)])

        # ---- Phase E: FFN for one lch (W2 resident, 2-bank sweeps) ---------
        # For lch=0 (which overlaps attention lch=1), the pre-gelu z is
        # stashed in SBUF and gelu + final add run in the tail, keeping the
        # ScalarE exp table loaded throughout attention.
        zst = kvp.tile([P, 4, D], bf16, tag="zst")

        def phase_e(lch, w2_sb, defer):
            uT = lnu.tile([P, KI, 512], bf16, tag="lnu", name=f"uT{lch}")
            for mt in range(KI):
                w1t = w1p.tile([P, KD, P], bf16, tag="w1s", name=f"w1_{lch}_{mt}")
                nc.sync.dma_start(w1t[:], w1_d[:, :, ts(mt, P)])
                pu = gtile()
                for k in range(KD):
                    nc.tensor.matmul(pu[:], w1t[:, k, :], ln2T[:, k, ts(lch, 512)],
                                     start=(k == 0), stop=(k == KD - 1))
                nc.vector.tensor_scalar(out=uT[:, mt, :], in0=pu[:],
                                        scalar1=b1_sb[:, mt : mt + 1], scalar2=None,
                                        op0=OP.add)
            for tt in range(4):
                t = lch * 4 + tt
                osb = None if defer else xrp.tile([P, D], f32, tag="xl",
                                                  name=f"osb{lch}_{tt}")
                for ncol in range(2):
                    pz = gtile(384)
                    for mt in range(KI):
                        nc.tensor.matmul(pz[:, :384],
                                         uT[:, mt, ts(tt, P)],
                                         w2_sb[:, mt, ts(ncol, 384)],
                                         start=(mt == 0), stop=(mt == KI - 1))
                    if defer:
                        nc.vector.tensor_tensor(out=zst[:, tt, ts(ncol, 384)],
                                                in0=pz[:, :384],
                                                in1=b2_sb[:, ts(ncol, 384)], op=OP.add)
                    else:
                        zb = tp.tile([P, 384], f32, tag="zb")
                        nc.vector.tensor_tensor(out=zb[:], in0=pz[:, :384],
                                                in1=b2_sb[:, ts(ncol, 384)], op=OP.add)
                        gt = tp.tile([P, 384], f32, tag="zb",
                                     name=f"gt{lch}_{tt}_{ncol}")
                        nc.scalar.activation(out=gt[:], in_=zb[:], func=AF.Gelu)
                        nc.vector.tensor_tensor(out=osb[:, ts(ncol, 384)], in0=gt[:],
                                                in1=out1[:, t, ts(ncol, 384)], op=OP.add)
                if not defer:
                    nc.sync.dma_start(out_d[t], osb[:])

        def flush_deferred():
            for tt in range(4):
                osb = xrp.tile([P, D], f32, tag="xl", name=f"osbd{tt}")
                for ncol in range(2):
                    gt = tp.tile([P, 384], f32, tag="zb", name=f"gtd{tt}_{ncol}")
                    nc.scalar.activation(out=gt[:], in_=zst[:, tt, ts(ncol, 384)],
                                         func=AF.Gelu)
                    nc.vector.tensor_tensor(out=osb[:, ts(ncol, 384)], in0=gt[:],
                                            in1=out1[:, tt, ts(ncol, 384)], op=OP.add)
                nc.sync.dma_start(out_d[tt], osb[:])

        # ---- main schedule -------------------------------------------------
        # attention lch=1 is emitted BEFORE D/E(0) so it has scheduler
        # priority: scores/exp keep ScalarE fed while D/E(0) matmuls fill the
        # PE gaps left by exp waits.
        for j in range(NPAIR):
            qk_proj(j)
            attention(j, 0)
        # wqk + x staging done; free their SBUF for the resident W2.
        wstr.release()
        xp.release()
        w2p = tc.alloc_tile_pool(name="w2p", bufs=1)
        w2_sb = w2p.tile([P, KI, D], bf16, tag="w2r")
        nc.sync.dma_start(w2_sb[:], w2_d[:])
        wo_sb = wres.tile([P, KD, D], bf16, tag="wow")
        nc.sync.dma_start(wo_sb[:], wo_d[:])
        x8_sb = kvp.tile([P, 4, D], bf16, tag="x8")
        nc.sync.dma_start(x8_sb[:], x8_d[:].rearrange("t p d -> p t d"))
        for j in range(NPAIR):
            attention(j, 1)
        phase_d(0, wo_sb)
        phase_e(0, w2_sb, defer=True)
        flush_deferred()
        phase_d(1, wo_sb)
        phase_e(1, w2_sb, defer=False)
        w2p.release()

    nc.compile()
    return nc


def _prep_host(x, attention_mask, ln1_g, ln1_b, Wqkv, bqkv, Wo, bo,
               ln2_g, ln2_b, W1, b1, W2, b2):
    x = _f32(x); mask = np.asarray(attention_mask)
    ln1_g = _f32(ln1_g); ln1_b = _f32(ln1_b)
    Wqkv = _f32(Wqkv); bqkv = _f32(bqkv)
    Wo = _f32(Wo); bo = _f32(bo)
    ln2_g = _f32(ln2_g); ln2_b = _f32(ln2_b)
    W1 = _f32(W1); b1 = _f32(b1); W2 = _f32(W2); b2 = _f32(b2)

    base = np.arange(H)[:, None] * 3 * DH
    q_idx = (base + np.arange(DH)).ravel()
    k_idx = (base + DH + np.arange(DH)).ravel()
    v_idx = (base + 2 * DH + np.arange(DH)).ravel()

    scale = 1.0 / np.sqrt(DH)
    Wq = ln1_g[:, None] * Wqkv[:, q_idx] * scale
    Wk = ln1_g[:, None] * Wqkv[:, k_idx]
    Wv = ln1_g[:, None] * Wqkv[:, v_idx]
    bq = (bqkv[q_idx] + ln1_b @ Wqkv[:, q_idx]) * scale
    bk = bqkv[k_idx] + ln1_b @ Wqkv[:, k_idx]
    bv = bqkv[v_idx] + ln1_b @ Wqkv[:, v_idx]
    W1p = ln2_g[:, None] * W1
    b1p = b1 + ln2_b @ W1

    shared = {
        "wqk": _bf16(_wpm(np.concatenate([Wq, Wk], axis=1), KD)),
        "bqk": np.ascontiguousarray(
            np.concatenate([_pm(bq, KD), _pm(bk, KD)], axis=1)),
        "wv": _bf16(_wpm(Wv, KD)),
        "wo": _bf16(_wpm(Wo, KD)),
        "w1": _bf16(_wpm(W1p, KD)),
        "b1": _pm(b1p, KI),
        "w2": _bf16(_wpm(W2, KI)),
        "b2": _bf16(b2[None, :]),
    }

    in_maps = []
    for c in range(NCORES):
        b, half = c // 2, c % 2
        own = slice(half * LQ, (half + 1) * LQ)
        oth = slice((1 - half) * LQ, (2 - half) * LQ)
        xl = np.concatenate([x[b, own], x[b, oth]], axis=0)
        ml = np.concatenate([mask[b, own], mask[b, oth]], axis=0)
        mb = (ml.astype(np.float32) - 1.0) * 30.0 - 2.0
        m = dict(shared)
        xlb = xl + (bo + bv @ Wo)[None, :]
        m["xloc"] = np.ascontiguousarray(xlb.reshape(NT, P, D))
        m["x8"] = _bf16(xlb[512:1024].reshape(4, P, D))
        m["mbias"] = np.ascontiguousarray(mb.reshape(NT, P).T)
        in_maps.append(m)
    return in_maps


LAST_RESULT = None  # BassKernelResults of the most recent run (for profiling)
TRACE = False


def kernel(**inputs):
    global LAST_RESULT
    from concourse.bass_utils import run_bass_kernel_spmd

    use_mask = not bool(np.asarray(inputs["attention_mask"]).all())
    key = f"nc{int(use_mask)}"
    if key not in _CACHE:
        _CACHE[key] = build(use_mask)
    nc = _CACHE[key]

    in_maps = _prep_host(**inputs)
    res = run_bass_kernel_spmd(nc, in_maps, list(range(NCORES)), trace=TRACE)
    LAST_RESULT = res

    out = np.empty((B, L, D), np.float32)
    for c in range(NCORES):
        b, half = c // 2, c % 2
        o = res.results[c]["out"].reshape(LQ, D)
        out[b, half * LQ : (half + 1) * LQ] = o
    return out


# revision 36
# speedup vs baseline: 1.0490x; 1.0490x over previous
"""Trainium2 Bass kernel for a dense transformer encoder layer.

Contract: kernel(**inputs) takes FULL unsharded inputs (as produced by the
problem's setup_inputs) and returns the FULL output [B, L, D] float32.

Sharding: 8 cores, data-parallel over batch (4) x sequence-split (2).
Core c handles batch b=c//2, sequence half h=c%2 (1024 query rows), but
computes K/V over the full 2048 keys of its batch item (keys are rotated so
each core's own rows come first -> one identical SPMD program, per-core data
only). No collectives.

Host prep (layout only): fold ln1_g/ln1_b into Wqkv/bqkv, ln2_g/ln2_b into
W1/b1, fold 1/sqrt(dh) into Wq/bq, de-interleave Wqkv into [Q|K] (feature-
major outputs) and V (row-major output), cast weights to bf16.

v2 on-chip dataflow per core (all matmuls bf16 with fp32 PSUM accumulate):
  Phase A (per 128-row tile): LN1 (bn_stats) -> PE-transpose -> lnT, then V
    matmuls for that tile immediately (keeps HAM clock warm, overlaps DVE LN).
  QK proj for all 6 head pairs -> qT_all/kT_all resident in SBUF.
  Attention, query-chunk (lch=512 rows) outer, head-pair inner:
    per round of 2 key tiles: 4 score matmuls issued A,B,A,B -- heads A/B use
    PE row-groups {0,1} vs {2,3} (K=64), so pairs run CONCURRENTLY on the
    array (2x scores throughput). exp batched N=1024 per head on ScalarE.
    attnV via ones-augmented V (M=65) accumulating [65,512] per head.
    1/sumexp via DVE reciprocal_approx_fast (5x faster than reciprocal).
  After lch=0 attention: out-proj + residual + LN2 + FFN for those 512 rows
    are EMITTED next, so the Tile scheduler fills lch=1's exp-wait PE gaps
    with FFN/proj matmuls. W2 is SBUF-resident (loaded once) so FFN2 needs
    only 2 PSUM banks, leaving the attention 6-bank working set intact.
PSUM budget: scores quad 4 banks + attnV pair 2 banks + general pool 2.
"""

import numpy as np
import ml_dtypes

B, L, D, H, I = 4, 2048, 768, 12, 3072
DH = D // H            # 64
P = 128
LQ = L // 2            # 1024 query rows per core
NCORES = 8
EPS = 1e-5

KD = D // P            # 6   k-subtiles over D
KI = I // P            # 24  k-subtiles over I
NT = L // P            # 16  key tiles
NTQ = LQ // P          # 8   query tiles
NPAIR = H // 2         # 6   head pairs
VW = H * 80            # 960: fp8 vaug, per head 64 V cols + ones col + 15 pad

_CACHE = {}


def _bf16(a):
    return np.ascontiguousarray(np.asarray(a, np.float32).astype(ml_dtypes.bfloat16))


def _f32(a):
    return np.ascontiguousarray(np.asarray(a, np.float32))


def _pm(vec, k):
    """[k*128] -> [128, k] partition-major."""
    return np.ascontiguousarray(np.asarray(vec, np.float32).reshape(k, P).T)


def _wpm(w, k):
    """[k*128, M] -> [128, k, M] partition-major lhsT/rhs layout."""
    w = np.asarray(w)
    return np.ascontiguousarray(w.reshape(k, P, w.shape[1]).transpose(1, 0, 2))


def build(use_mask=False):
    import concourse.bass as bass
    import concourse.mybir as mybir
    import concourse.tile as tile
    from concourse import bacc
    from concourse.bass import ts
    from concourse.masks import make_identity
    from contextlib import ExitStack

    f32 = mybir.dt.float32
    bf16 = mybir.dt.bfloat16
    f8 = mybir.dt.float8e4
    PM = mybir.MatmulPerfMode
    AF = mybir.ActivationFunctionType
    OP = mybir.AluOpType

    nc = bacc.Bacc(None, target_bir_lowering=False, debug=False)

    # ---- DRAM I/O ----------------------------------------------------------
    x_d = nc.dram_tensor("xloc", [NT, P, D], f32, kind="ExternalInput")
    mb_d = nc.dram_tensor("mbias", [P, NT], f32, kind="ExternalInput")
    wqk_d = nc.dram_tensor("wqk", [P, KD, 2 * D], bf16, kind="ExternalInput")
    bqk_d = nc.dram_tensor("bqk", [P, 2 * KD], f32, kind="ExternalInput")
    wv_d = nc.dram_tensor("wv", [P, KD, D], bf16, kind="ExternalInput")
    bv_d = nc.dram_tensor("bv", [1, D], bf16, kind="ExternalInput")
    wo_d = nc.dram_tensor("wo", [P, KD, D], bf16, kind="ExternalInput")
    w1_d = nc.dram_tensor("w1", [P, KD, I], bf16, kind="ExternalInput")
    b1_d = nc.dram_tensor("b1", [P, KI], f32, kind="ExternalInput")
    w2_d = nc.dram_tensor("w2", [P, KI, D], bf16, kind="ExternalInput")
    b2_d = nc.dram_tensor("b2", [1, D], bf16, kind="ExternalInput")
    out_d = nc.dram_tensor("out", [NTQ, P, D], f32, kind="ExternalOutput")
    scr_d = nc.dram_tensor("warm_scr", [P, P], f32)

    with ExitStack() as ctx:
        tc = ctx.enter_context(tile.TileContext(nc))
        # PSUM: quad (4 banks) for paired scores, pvp (2) for attnV
        # accumulators, g (2) for everything else (V/QK/O/FFN/transposes).
        quadp = ctx.enter_context(tc.tile_pool(name="quadp", bufs=2, space="PSUM"))
        pvpp = ctx.enter_context(tc.tile_pool(name="pvpp", bufs=1, space="PSUM"))
        gp = ctx.enter_context(tc.tile_pool(name="gp", bufs=2, space="PSUM"))
        const = ctx.enter_context(tc.tile_pool(name="const", bufs=1))
        wres = ctx.enter_context(tc.tile_pool(name="wres", bufs=1))
        w1p = ctx.enter_context(tc.tile_pool(name="w1p", bufs=3))
        kvp = ctx.enter_context(tc.tile_pool(name="kvp", bufs=1))
        qkt = ctx.enter_context(tc.tile_pool(name="qkt", bufs=1))
        lnu = ctx.enter_context(tc.tile_pool(name="lnu", bufs=1))
        expp = ctx.enter_context(tc.tile_pool(name="expp", bufs=3))
        xrp = ctx.enter_context(tc.tile_pool(name="xrp", bufs=2))
        tp = ctx.enter_context(tc.tile_pool(name="tp", bufs=2))
        # released mid-build to make room for the resident W2
        xp = tc.alloc_tile_pool(name="xp", bufs=2)
        wstr = tc.alloc_tile_pool(name="wstr", bufs=3)

        nname = [0]

        def gtile(cols=512, dt=f32):
            nname[0] += 1
            return gp.tile([P, cols], dt, tag="g", name=f"g{nname[0]}")

        # first x tile pair + V weights go to the DMA queue ahead of the
        # slow [1,D]->[P,D] broadcast const loads (the first LN was
        # otherwise stuck behind ~12us of replication DMA).
        xt0 = xp.tile([P, 2, D], f32, tag="xl", name="xt0")
        nc.sync.dma_start(xt0[:, 0, :], x_d[0])
        nc.sync.dma_start(xt0[:, 1, :], x_d[1])
        wv_early = wres.tile([P, KD, D], bf16, tag="wow", name="wv_early")
        nc.sync.dma_start(wv_early[:], wv_d[:])

        # ---- constants -----------------------------------------------------
        ident = const.tile([P, P], bf16, tag="ident")
        make_identity(nc, ident)
        epst = const.tile([P, 1], f32, tag="eps")
        nc.vector.memset(epst, EPS)
        u32 = mybir.dt.uint32
        shift1 = const.tile([P, 1], u32, tag="sh1")
        nc.vector.memset(shift1, 1)
        magic = const.tile([P, 4], u32, tag="magic")
        nc.vector.memset(magic, 0x5F3759DF)
        neg2 = const.tile([P, 1], f32, tag="neg2")
        nc.vector.memset(neg2, -2.0)
        mbias = const.tile([P, NT], f32, tag="mb")
        nc.sync.dma_start(mbias[:], mb_d[:])
        bqk_sb = const.tile([P, 2 * KD], f32, tag="bqk")
        nc.sync.dma_start(bqk_sb[:], bqk_d[:])
        bv_row = const.tile([1, D], bf16, tag="bvr")
        nc.sync.dma_start(bv_row[:], bv_d[:])
        bv_sb = const.tile([P, D], bf16, tag="bv")
        nc.gpsimd.partition_broadcast(bv_sb[:], bv_row[:])
        b1_sb = const.tile([P, KI], f32, tag="b1")
        nc.sync.dma_start(b1_sb[:], b1_d[:])
        b2_row = const.tile([1, D], bf16, tag="b2r")
        nc.sync.dma_start(b2_row[:], b2_d[:])
        b2_sb = const.tile([P, D], bf16, tag="b2")
        nc.gpsimd.partition_broadcast(b2_sb[:], b2_row[:])

        # persistent activations
        lnT = lnu.tile([P, KD, L], bf16, tag="lnu")        # [768, 2048] transposed LN1
        vaug = kvp.tile([P, NT, VW], f8, tag="vo")         # fp8 V row-major + ones cols
        aoT = kvp.tile([P, KD, LQ], bf16, tag="aoT")       # attn out, feature-major
        out1 = kvp.tile([P, NTQ, D], bf16, tag="out1")     # attn residual stream
        ln2T = kvp.tile([P, KD, LQ], bf16, tag="ln2T")
        qT_all = qkt.tile([P, NPAIR, LQ], bf16, tag="qTa")
        kT_all = qkt.tile([P, NPAIR, L], bf16, tag="kTa")

        def layernorm(dst_bf16, src, stats_tag):
            """dst = (src - mean)/sqrt(var+eps) over free dim 768."""
            view = src.rearrange("p (a b) -> p a b", b=256)
            stats = tp.tile([P, 3, 6], f32, tag=stats_tag + "s")
            mv = tp.tile([P, 2], f32, tag=stats_tag + "m")
            for i in range(3):
                nc.vector.bn_stats(out=stats[:, i, :], in_=view[:, i, :])
            nc.vector.bn_aggr(out=mv[:], in_=stats[:])
            # mv[:,1] = 1/sqrt(var+eps)
            nc.scalar.activation(out=mv[:, 1:2], in_=mv[:, 1:2], func=AF.Sqrt,
                                 bias=epst[:], scale=1.0)
            nc.vector.reciprocal(out=mv[:, 1:2], in_=mv[:, 1:2])
            nc.vector.tensor_scalar(out=dst_bf16, in0=src,
                                    scalar1=mv[:, 0:1], scalar2=mv[:, 1:2],
                                    op0=OP.subtract, op1=OP.mult)

        def transpose_128(dst, src_bf16):
            """dst[128,128] (sbuf bf16) = src.T via PE."""
            nname[0] += 1
            pt = gp.tile([P, P], bf16, tag="g", name=f"pt{nname[0]}")
            nc.tensor.transpose(pt[:], src_bf16, ident[:])
            nc.vector.tensor_copy(out=dst, in_=pt[:])

        # ---- Phase A: LN1 + transpose -> lnT, V per tile -------------------
        wv_sb = wv_early
        vview = vaug.rearrange("p t (h c) -> p t h c", c=80)
        nc.vector.memset(vview[:, :, :, DH : 80], 0.0)
        nc.vector.memset(vview[:, :, :, DH : DH + 1], 1.0)
        bv3 = bv_sb.rearrange("p (h c) -> p h c", c=DH)

        def v_tile(t):
            for ncol in range(2):
                pv = gtile(384)
                for k in range(KD):
                    nc.tensor.matmul(pv[:, :384], lnT[:, k, ts(t, P)],
                                     wv_sb[:, k, ts(ncol, 384)],
                                     start=(k == 0), stop=(k == KD - 1))
                dst = vview[:, t, 6 * ncol : 6 * ncol + 6, 0:DH]
                src = pv[:, :384].rearrange("p (h c) -> p h c", c=DH)
                bvb = bv3[:, 6 * ncol : 6 * ncol + 6, :]
                nc.vector.tensor_tensor(out=dst, in0=src, in1=bvb, op=OP.add)

        # HAM warm-up: ~3.5us of real matmuls first thing flips the PE clock
        # gate to 8/8 before the transposes (HAM-invisible) start.
        wps = gtile(P)
        for w in range(40):
            nc.tensor.matmul(wps[:, 0:P], ident[:], ident[:],
                             start=(w == 0), stop=(w == 39))
        wsb = tp.tile([P, P], f32, tag="wsb")
        nc.vector.tensor_copy(out=wsb[:], in_=wps[:, 0:P])
        nc.sync.dma_start(scr_d[:], wsb[:])

        for tpair in range(NT // 2):
            if tpair == 0:
                xt = xt0
            else:
                xt = xp.tile([P, 2, D], f32, tag="xl")
                nc.sync.dma_start(xt[:], x_d[2 * tpair : 2 * tpair + 2].rearrange("t p d -> p t d"))
            for s in range(2):
                t = 2 * tpair + s
                lnbf = tp.tile([P, D], bf16, tag="lnbf")
                layernorm(lnbf[:], xt[:, s, :], "ln1")
                for j in range(KD):
                    transpose_128(lnT[:, j, ts(t, P)], lnbf[:, ts(j, P)])
                v_tile(t)

        # ---- QK projection for one pair -> resident qT_all/kT_all ----------
        def qk_proj(j):
            wqkj = wstr.tile([P, KD, 2 * P], bf16, tag="wqkj")
            nc.sync.dma_start(wqkj[:, :, 0:P], wqk_d[:, :, ts(j, P)])
            nc.sync.dma_start(wqkj[:, :, P : 2 * P], wqk_d[:, :, D + j * P : D + (j + 1) * P])
            for lch in range(2):
                pq = gtile()
                for k in range(KD):
                    nc.tensor.matmul(pq[:], wqkj[:, k, 0:P], lnT[:, k, ts(lch, 512)],
                                     start=(k == 0), stop=(k == KD - 1))
                nc.vector.tensor_scalar(out=qT_all[:, j, ts(lch, 512)], in0=pq[:],
                                        scalar1=bqk_sb[:, j : j + 1], scalar2=None,
                                        op0=OP.add)
            for nch in range(4):
                pk = gtile()
                for k in range(KD):
                    nc.tensor.matmul(pk[:], wqkj[:, k, P : 2 * P], lnT[:, k, ts(nch, 512)],
                                     start=(k == 0), stop=(k == KD - 1))
                nc.vector.tensor_scalar(out=kT_all[:, j, ts(nch, 512)], in0=pk[:],
                                        scalar1=bqk_sb[:, KD + j : KD + j + 1],
                                        scalar2=None, op0=OP.add)

        # ---- attention for (pair j, query chunk lch) -----------------------
        # One round = one key tile: both heads' score matmuls (concurrent on
        # PE row-groups {0,1}/{2,3}) land in one 2-bank quad tile, and a
        # single N=1024 exp covers both heads. quadp bufs=2 double-buffers
        # rounds so scores(r+1) only waits on exp(r-1) -- the exp stream on
        # ScalarE runs gap-free.
        def attention(j, lch):
            pvp = pvpp.tile([P, 2, 512], f32, tag="pvp", name=f"pv{j}_{lch}")
            wd = gtile(P) if lch == 0 else None
            for mtp in range(NT // 2):
                if lch == 0 and 1 <= mtp <= 6:
                    for w in range(8):
                        nc.tensor.matmul(wd[:, 0:P], ident[:], ident[:],
                                         start=(mtp == 1 and w == 0),
                                         stop=(mtp == 6 and w == 7))
                expT2 = expp.tile([P, 2, 2, 512], f8, tag="expT",
                                  name=f"ex{j}_{lch}_{mtp}")
                for s in range(2):
                    mt = 2 * mtp + s
                    dq = quadp.tile([P, 2, 512], f32, tag="quad",
                                    name=f"qd{j}_{lch}_{mt}")
                    for hh in range(2):
                        r = hh * 64
                        nc.tensor.matmul(dq[:, hh, :],
                                         kT_all[r : r + 64, j, ts(mt, P)],
                                         qT_all[r : r + 64, j, ts(lch, 512)],
                                         start=True, stop=True)
                    # exp(s - 2): the -2 keeps exp within fp8e4 range; the
                    # softmax ratio is shift-invariant.
                    if use_mask:
                        for hh in range(2):
                            nc.scalar.activation(out=expT2[:, s, hh, :],
                                                 in_=dq[:, hh, :], func=AF.Exp,
                                                 bias=mbias[:, mt : mt + 1],
                                                 scale=1.0)
                    else:
                        nc.scalar.activation(out=expT2[:, s, :, :], in_=dq[:],
                                             func=AF.Exp, bias=neg2[:])
                for hh in range(2):
                    h = 2 * j + hh
                    nc.tensor.matmul(
                        pvp[0:80, hh, :],
                        vaug[:, 2 * mtp : 2 * mtp + 2, 80 * h : 80 * h + 80],
                        expT2[:, :, hh, :],
                        perf_mode=PM.DoubleRow,
                        start=(mtp == 0), stop=(mtp == NT // 2 - 1))
            if lch == 0:
                wsb2 = tp.tile([P, P], f32, tag="wsb")
                nc.vector.tensor_copy(out=wsb2[:], in_=wd[:, 0:P])
                nc.sync.dma_start(scr_d[:], wsb2[:])
            for hh in range(2):
                r = hh * 64
                rr0 = tp.tile([1, 512], f32, tag="rr0", bufs=2)
                nc.vector.tensor_copy(out=rr0[:], in_=pvp[DH : DH + 1, hh, :])
                rr = tp.tile([1, 512], f32, tag="rr", bufs=2)
                nc.vector.reciprocal_approx_fast(out=rr[:], in_=rr0[:])
                rrb = tp.tile([64, 512], f32, tag="rrb", bufs=2)
                nc.gpsimd.partition_broadcast(rrb[:], rr[:])
                nc.vector.tensor_tensor(out=aoT[r : r + 64, j, ts(lch, 512)],
                                        in0=pvp[0:DH, hh, :],
                                        in1=rrb[:], op=OP.mult)

        # ---- Phase D: out-proj + residual + LN2 + transpose (4 tiles) ------
        # LN2 rstd is computed on DVE only (fast-inverse-sqrt bit trick + 2
        # Newton steps, batched over the 4 tiles) so no ScalarE table switch
        # lands inside the attention exp stream.
        def phase_d(lch, wo_sb):
            mv4 = tp.tile([P, 4, 2], f32, tag="mv4", name=f"mv4_{lch}")
            for tt in range(4):
                t = lch * 4 + tt
                xr = xrp.tile([P, D], f32, tag="xl")
                nc.sync.dma_start(xr[:], x_d[t].rearrange("p d -> p d"))
                for ncol in range(2):
                    po = gtile(384)
                    for k in range(KD):
                        nc.tensor.matmul(po[:, :384], aoT[:, k, ts(t, P)],
                                         wo_sb[:, k, ts(ncol, 384)],
                                         start=(k == 0), stop=(k == KD - 1))
                    nc.vector.tensor_tensor(out=out1[:, t, ts(ncol, 384)],
                                            in0=po[:, :384],
                                            in1=xr[:, ts(ncol, 384)], op=OP.add)
                view = out1[:, t, :].rearrange("p (a b) -> p a b", b=256)
                stats = tp.tile([P, 3, 6], f32, tag="ln2s")
                for i in range(3):
                    nc.vector.bn_stats(out=stats[:, i, :], in_=view[:, i, :])
                nc.vector.bn_aggr(out=mv4[:, tt, :], in_=stats[:])
            # rstd for all 4 tiles: r = rsqrt(var + eps) without ScalarE
            vr = tp.tile([P, 4], f32, tag="vr", name=f"vr{lch}")
            nc.vector.tensor_scalar(out=vr[:], in0=mv4[:, :, 1],
                                    scalar1=epst[:], scalar2=None, op0=OP.add)
            y0 = tp.tile([P, 4], f32, tag="y0", name=f"y0{lch}")
            nc.vector.tensor_scalar(out=y0.bitcast(u32)[:], in0=vr.bitcast(u32)[:],
                                    scalar1=shift1[:], scalar2=None,
                                    op0=OP.logical_shift_right)
            nc.vector.tensor_tensor(out=y0.bitcast(u32)[:], in0=magic[:],
                                    in1=y0.bitcast(u32)[:], op=OP.subtract)
            for it in range(2):
                aa = tp.tile([P, 4], f32, tag="nsq", name=f"nsq{lch}_{it}")
                nc.vector.tensor_tensor(out=aa[:], in0=y0[:], in1=y0[:], op=OP.mult)
                nc.vector.tensor_tensor(out=aa[:], in0=aa[:], in1=vr[:], op=OP.mult)
                nc.vector.tensor_scalar(out=aa[:], in0=aa[:],
                                        scalar1=-0.5, scalar2=1.5,
                                        op0=OP.mult, op1=OP.add)
                nc.vector.tensor_tensor(out=y0[:], in0=y0[:], in1=aa[:], op=OP.mult)
            for tt in range(4):
                t = lch * 4 + tt
                lnbf = tp.tile([P, D], bf16, tag="lnbf")
                nc.vector.tensor_scalar(out=lnbf[:], in0=out1[:, t, :],
                                        scalar1=mv4[:, tt, 0:1], scalar2=y0[:, tt : tt + 1],
                                        op0=OP.subtract, op1=OP.mult)
                for k in range(KD):
                    transpose_128(ln2T[:, k, ts(t, P)], lnbf[:, ts(k, P)])

        # ---- Phase E: FFN for one lch (W2 resident, 2-bank sweeps) ---------
        # For lch=0 (which overlaps attention lch=1), the pre-gelu z is
        # stashed in SBUF and gelu + final add run in the tail, keeping the
        # ScalarE exp table loaded throughout attention.
        zst = kvp.tile([P, 4, D], bf16, tag="zst")

        def phase_e(lch, w2_sb, defer):
            uT = lnu.tile([P, KI, 512], bf16, tag="lnu", name=f"uT{lch}")
            for mt in range(KI):
                w1t = w1p.tile([P, KD, P], bf16, tag="w1s", name=f"w1_{lch}_{mt}")
                nc.sync.dma_start(w1t[:], w1_d[:, :, ts(mt, P)])
                pu = gtile()
                for k in range(KD):
                    nc.tensor.matmul(pu[:], w1t[:, k, :], ln2T[:, k, ts(lch, 512)],
                                     start=(k == 0), stop=(k == KD - 1))
                nc.vector.tensor_scalar(out=uT[:, mt, :], in0=pu[:],
                                        scalar1=b1_sb[:, mt : mt + 1], scalar2=None,
                                        op0=OP.add)
            for tt in range(4):
                t = lch * 4 + tt
                osb = None if defer else xrp.tile([P, D], f32, tag="xl",
                                                  name=f"osb{lch}_{tt}")
                for ncol in range(2):
                    pz = gtile(384)
                    for mt in range(KI):
                        nc.tensor.matmul(pz[:, :384],
                                         uT[:, mt, ts(tt, P)],
                                         w2_sb[:, mt, ts(ncol, 384)],
                                         start=(mt == 0), stop=(mt == KI - 1))
                    if defer:
                        nc.vector.tensor_tensor(out=zst[:, tt, ts(ncol, 384)],
                                                in0=pz[:, :384],
                                                in1=b2_sb[:, ts(ncol, 384)], op=OP.add)
                    else:
                        zb = tp.tile([P, 384], f32, tag="zb")
                        nc.vector.tensor_tensor(out=zb[:], in0=pz[:, :384],
                                                in1=b2_sb[:, ts(ncol, 384)], op=OP.add)
                        gt = tp.tile([P, 384], f32, tag="zb",
                                     name=f"gt{lch}_{tt}_{ncol}")
                        nc.scalar.activation(out=gt[:], in_=zb[:], func=AF.Gelu)
                        nc.vector.tensor_tensor(out=osb[:, ts(ncol, 384)], in0=gt[:],
                                                in1=out1[:, t, ts(ncol, 384)], op=OP.add)
                if not defer:
                    nc.sync.dma_start(out_d[t], osb[:])

        def flush_deferred():
            for tt in range(4):
                osb = xrp.tile([P, D], f32, tag="xl", name=f"osbd{tt}")
                for ncol in range(2):
                    gt = tp.tile([P, 384], f32, tag="zb", name=f"gtd{tt}_{ncol}")
                    nc.scalar.activation(out=gt[:], in_=zst[:, tt, ts(ncol, 384)],
                                         func=AF.Gelu)
                    nc.vector.tensor_tensor(out=osb[:, ts(ncol, 384)], in0=gt[:],
                                            in1=out1[:, tt, ts(ncol, 384)], op=OP.add)
                nc.sync.dma_start(out_d[tt], osb[:])

        # ---- main schedule -------------------------------------------------
        # attention lch=1 is emitted BEFORE D/E(0) so it has scheduler
        # priority: scores/exp keep ScalarE fed while D/E(0) matmuls fill the
        # PE gaps left by exp waits.
        for j in range(NPAIR):
            qk_proj(j)
            attention(j, 0)
        # wqk + x staging done; free their SBUF for the resident W2.
        wstr.release()
        xp.release()
        w2p = tc.alloc_tile_pool(name="w2p", bufs=1)
        w2_sb = w2p.tile([P, KI, D], bf16, tag="w2r")
        nc.sync.dma_start(w2_sb[:], w2_d[:])
        wo_sb = wres.tile([P, KD, D], bf16, tag="wow")
        nc.sync.dma_start(wo_sb[:], wo_d[:])
        for j in range(NPAIR):
            attention(j, 1)
        phase_d(0, wo_sb)
        phase_e(0, w2_sb, defer=True)
        # D(1) before the flush: its xr tiles rotate the "xl" tag ahead of
        # the flush's osb tiles, so the x reload DMAs start during attention
        # instead of waiting on flush output DMAs (which gate on the last
        # exp). Flush gelus still queue after all exps on the ScalarE FIFO.
        phase_d(1, wo_sb)
        flush_deferred()
        phase_e(1, w2_sb, defer=False)
        w2p.release()

    nc.compile()
    return nc


def _prep_host(x, attention_mask, ln1_g, ln1_b, Wqkv, bqkv, Wo, bo,
               ln2_g, ln2_b, W1, b1, W2, b2):
    x = _f32(x); mask = np.asarray(attention_mask)
    ln1_g = _f32(ln1_g); ln1_b = _f32(ln1_b)
    Wqkv = _f32(Wqkv); bqkv = _f32(bqkv)
    Wo = _f32(Wo); bo = _f32(bo)
    ln2_g = _f32(ln2_g); ln2_b = _f32(ln2_b)
    W1 = _f32(W1); b1 = _f32(b1); W2 = _f32(W2); b2 = _f32(b2)

    base = np.arange(H)[:, None] * 3 * DH
    q_idx = (base + np.arange(DH)).ravel()
    k_idx = (base + DH + np.arange(DH)).ravel()
    v_idx = (base + 2 * DH + np.arange(DH)).ravel()

    scale = 1.0 / np.sqrt(DH)
    Wq = ln1_g[:, None] * Wqkv[:, q_idx] * scale
    Wk = ln1_g[:, None] * Wqkv[:, k_idx]
    Wv = ln1_g[:, None] * Wqkv[:, v_idx]
    bq = (bqkv[q_idx] + ln1_b @ Wqkv[:, q_idx]) * scale
    bk = bqkv[k_idx] + ln1_b @ Wqkv[:, k_idx]
    bv = bqkv[v_idx] + ln1_b @ Wqkv[:, v_idx]
    W1p = ln2_g[:, None] * W1
    b1p = b1 + ln2_b @ W1

    shared = {
        "wqk": _bf16(_wpm(np.concatenate([Wq, Wk], axis=1), KD)),
        "bqk": np.ascontiguousarray(
            np.concatenate([_pm(bq, KD), _pm(bk, KD)], axis=1)),
        "wv": _bf16(_wpm(Wv, KD)),
        "bv": _bf16(bv[None, :]),
        "wo": _bf16(_wpm(Wo, KD)),
        "w1": _bf16(_wpm(W1p, KD)),
        "b1": _pm(b1p, KI),
        "w2": _bf16(_wpm(W2, KI)),
        "b2": _bf16(b2[None, :]),
    }

    in_maps = []
    for c in range(NCORES):
        b, half = c // 2, c % 2
        own = slice(half * LQ, (half + 1) * LQ)
        oth = slice((1 - half) * LQ, (2 - half) * LQ)
        xl = np.concatenate([x[b, own], x[b, oth]], axis=0)
        ml = np.concatenate([mask[b, own], mask[b, oth]], axis=0)
        mb = (ml.astype(np.float32) - 1.0) * 30.0 - 2.0
        m = dict(shared)
        m["xloc"] = np.ascontiguousarray((xl + bo[None, :]).reshape(NT, P, D))
        m["mbias"] = np.ascontiguousarray(mb.reshape(NT, P).T)
        in_maps.append(m)
    return in_maps


LAST_RESULT = None  # BassKernelResults of the most recent run (for profiling)
TRACE = False


def kernel(**inputs):
    global LAST_RESULT
    from concourse.bass_utils import run_bass_kernel_spmd

    use_mask = not bool(np.asarray(inputs["attention_mask"]).all())
    key = f"nc{int(use_mask)}"
    if key not in _CACHE:
        _CACHE[key] = build(use_mask)
    nc = _CACHE[key]

    in_maps = _prep_host(**inputs)
    res = run_bass_kernel_spmd(nc, in_maps, list(range(NCORES)), trace=TRACE)
    LAST_RESULT = res

    out = np.empty((B, L, D), np.float32)
    for c in range(NCORES):
        b, half = c // 2, c % 2
        o = res.results[c]["out"].reshape(LQ, D)
        out[b, half * LQ : (half + 1) * LQ] = o
    return out
